# revision 23
# baseline (speedup 1.0000x reference)
"""Trainium2 Bass kernel for a (buggy-but-well-defined) ConvTranspose2d.

Math (matches the reference exactly):
  out[b, co, i, j] = sum_{ci,kh,kw} ker[ci,co,3-kh,3-kw] * xpad[b,ci,i+kh,j+kw]
                     + bias_sum * cnt[i] * cnt[j]          for i,j in [0,66)
  out is zero elsewhere in the (B,128,126,126) output.
  xpad = x[:, :, :63, :63] zero-padded by 3 on every side -> (69,69).
  cnt  = conv(ones(63), ones(4)) = [1,2,3,4,...,4,3,2,1]  (len 66)

Strategy: data-parallel over batch (2 items / core on 8 cores).  Per core,
16 shifted 128x128 matmuls (contraction over ci on the partition dim)
accumulate each group of <=7 output rows (N = R*66 <= 462) into one PSUM
bank, plus one rank-1 K=1 matmul that adds the bias field.  Matmuls run in
the fp32r dtype (fp32 with the mantissa RNE-rounded to 11 bits — the PE's
single-pass fp32 mode, 4x the throughput of plain fp32); operands are
pre-rounded to the fp32r encoding host-side so they can be DMA'd straight
into fp32r SBUF tiles.  Weights and the padded x are shipped as one merged,
host-prepared tensor so each matmul depends on a single DMA; the mostly-zero
full output is assembled host-side.
"""

import numpy as np

import concourse.bacc as bacc
import concourse.mybir as mybir
import concourse.tile as tile
from concourse.bass_utils import run_bass_kernel_spmd

B, CIN, COUT, K, H, W = 16, 128, 128, 4, 64, 64
NCORES = 8
BPC = B // NCORES          # batch items per core
HV = H - 1                 # 63 valid input rows/cols
HP = HV + 2 * (K - 1)      # 69 padded
HO = HV + K - 1            # 66 output rows/cols (nonzero region)
HOUT = (H - 1) * 2         # 126 full output rows/cols
NWT = K * K * COUT         # 2048 weight cols
NXP = HP * HP              # 4761 padded-image cols per batch item
NXW = NWT + BPC * NXP      # merged wt+xpad tensor cols
NBF = HO * HO + COUT       # bias-field input: 66*66 field + 128 ones
F32 = mybir.dt.float32
F32R = mybir.dt.float32r

# Output row groups: (start_row, n_rows).  Grouped in two halves of 5 so at
# most 5 PSUM accumulation groups are live at once and each tap's weights are
# reused across 5 consecutive matmuls.  All N = R*66 >= 256 (full-rate f32r).
GROUPS = [(0, 7), (7, 7), (14, 7), (21, 7), (28, 5),
          (33, 7), (40, 7), (47, 7), (54, 7), (61, 5)]

_CACHE = {}


def _build_nc():
    # Bacc (not raw Bass): its finalize() legalizes sync waits — moving
    # excess matmul waits onto LDWEIGHTS and splitting multi-waits onto
    # EventSemaphore instructions — which walrus codegen requires.
    nc = bacc.Bacc(None)
    xw = nc.dram_tensor("xw", [CIN, NXW], F32R, kind="ExternalInput")
    bf = nc.dram_tensor("bf", [NBF], F32R, kind="ExternalInput")
    out = nc.dram_tensor("out", [BPC, COUT, HO, HO], F32, kind="ExternalOutput")

    with tile.TileContext(nc) as tc:
        with (
            tc.tile_pool(name="xwpool", bufs=1) as xwpool,
            tc.tile_pool(name="cpool", bufs=1) as cpool,
            tc.tile_pool(name="acc", bufs=8, space="PSUM") as psum_pool,
            tc.tile_pool(name="opool", bufs=4) as opool,
        ):
            xwt = xwpool.tile([CIN, NXW], F32R)
            # Chunked input load so the first matmuls start as soon as the
            # weights + the first half of batch 0's image have landed:
            # [weights | b0 rows 0..38 | b0 rows 39..68 | b1 image].
            c1 = NWT + 39 * HP
            nc.sync.dma_start(xwt[:, :NWT], xw[:, :NWT])
            nc.sync.dma_start(xwt[:, NWT:c1], xw[:, NWT:c1])
            nc.sync.dma_start(xwt[:, c1:NWT + NXP], xw[:, c1:NWT + NXP])
            nc.sync.dma_start(xwt[:, NWT + NXP:], xw[:, NWT + NXP:])

            bft = cpool.tile([1, NBF], F32R)
            nc.sync.dma_start(bft[:1, :], bf[None, :])
            ones = bft[0:1, HO * HO:]

            xv = xwt[:, NWT:].rearrange("p (b h w) -> p b h w",
                                        b=BPC, h=HP, w=HP)

            for b in range(BPC):
                for half in range(2):
                    groups = GROUPS[half * 5:(half + 1) * 5]
                    ptiles = {}
                    for i0, r in groups:
                        ptiles[i0] = psum_pool.tile([COUT, 7 * HO], F32,
                                                    tag="acc", name="acc")
                    for t in range(K * K):
                        kh, kw = divmod(t, K)
                        lhsT = xwt[:, t * COUT:(t + 1) * COUT]
                        for i0, r in groups:
                            rhs = xv[:, b, i0 + kh:i0 + kh + r, kw:kw + HO]
                            nc.tensor.matmul(ptiles[i0][:, :r * HO], lhsT, rhs,
                                             start=(t == 0), stop=False)
                            if t == K * K - 1:
                                # Close the group immediately after its last
                                # tap so the PSUM->SBUF copy and out-DMA of
                                # early groups overlap the remaining matmuls.
                                rb = bft[0:1, i0 * HO:(i0 + r) * HO]
                                nc.tensor.matmul(ptiles[i0][:, :r * HO], ones,
                                                 rb, start=False, stop=True)
                                otile = opool.tile([COUT, 7 * HO], F32,
                                                   tag="ot", name="ot")
                                nc.vector.tensor_copy(otile[:, :r * HO],
                                                      ptiles[i0][:, :r * HO])
                                nc.sync.dma_start(out[b, :, i0:i0 + r, :],
                                                  otile[:, :r * HO])
    nc.finalize()
    return nc


def get_nc():
    if "nc" not in _CACHE:
        _CACHE["nc"] = _build_nc()
    return _CACHE["nc"]


def _fp32r(a):
    """RNE-round fp32 -> the PE's fp32r encoding (11-bit mantissa, same 4B).

    Bit-exact with libwalrus's fp32_to_fp32r (verified on 2e5 random values).
    """
    u = np.ascontiguousarray(a, dtype=np.float32).view(np.uint32)
    r = (u + np.uint32(0x7FF) + ((u >> np.uint32(12)) & np.uint32(1))) \
        & np.uint32(0xFFFFF000)
    return r.view(np.float32)


def prep_inputs(x, kernel, bias):
    """Host-side prep: per-core input maps (numpy only, negligible cost)."""
    x = _fp32r(np.asarray(x, dtype=np.float32))
    ker = np.asarray(kernel, dtype=np.float32)
    bias = np.asarray(bias, dtype=np.float32)

    kf = ker[:, :, ::-1, ::-1]                        # [ci, co, kh, kw] flipped
    wt = _fp32r(np.ascontiguousarray(kf.transpose(0, 2, 3, 1)).reshape(
        CIN, NWT))                                    # [ci, (kh kw co)]

    cnt = np.convolve(np.ones(HV, np.float32), np.ones(K, np.float32))
    bias_sum = np.sum(bias[:COUT], dtype=np.float32)
    bfield = np.empty(NBF, np.float32)
    bfield[:HO * HO] = (bias_sum * np.outer(cnt, cnt)).astype(np.float32).ravel()
    bfield[HO * HO:] = 1.0
    bfield = _fp32r(bfield)

    in_maps = []
    for c in range(NCORES):
        xw = np.zeros((CIN, NXW), np.float32)
        xw[:, :NWT] = wt
        xp = xw[:, NWT:].reshape(CIN, BPC, HP, HP)
        # x is already fp32r-rounded; zeros are fp32r-clean.
        xp[:, :, K - 1:K - 1 + HV, K - 1:K - 1 + HV] = \
            x[c * BPC:(c + 1) * BPC, :, :HV, :HV].transpose(1, 0, 2, 3)
        in_maps.append({"xw": xw, "bf": bfield})
    return in_maps


def assemble(per_core_outs):
    out = np.zeros((B, COUT, HOUT, HOUT), np.float32)
    for c, o in enumerate(per_core_outs):
        out[c * BPC:(c + 1) * BPC, :, :HO, :HO] = o
    return out


def run(inputs, **spmd_kwargs):
    """Returns (full_output, BassKernelResults)."""
    nc = get_nc()
    in_maps = prep_inputs(**inputs)
    res = run_bass_kernel_spmd(nc, in_maps, list(range(NCORES)), **spmd_kwargs)
    return assemble([r["out"] for r in res.results]), res


def kernel(**inputs):
    out, _ = run(inputs)
    return out
